# revision 1
# baseline (speedup 1.0000x reference)
"""YOLO detection-layer loss (nn_DetectionLayerNoCuda) on 8 trn2 NeuronCores.

Math: the six losses depend on x only at the ~320 GT-assigned cells (plus a
closed-form count term for the non-object CrossEntropy cells), so the kernel
gathers one 255-channel column per ground-truth box with a data-dependent
indirect DMA (indices computed on device from y_true), computes IoU/argmax/
targets/losses on device, and reduces to 6 partial sums per core.

Sharding: pure data parallel over batch — core c handles images [2c, 2c+1]
(20 GTs each, 40 per core). Host passes each core its batch shard in
channels-last layout ([b, h, w, c] -> [11552, 255]) so a GT's 255 channels are
one contiguous row; host sums the 8 per-core 6-vectors (all-reduce on host).
"""
import sys
import types

import numpy as np

BS = 16
GS = 76
N_GT = 20
N_ANCH = 3
N_CLS = 80
N_ATTR = 85
N_CH = N_ANCH * N_ATTR  # 255
N_CORES = 8
B_PER_CORE = BS // N_CORES  # 2
G_PER_CORE = B_PER_CORE * N_GT  # 40
ROWS = B_PER_CORE * GS * GS  # 11552
CELLS_PER_CORE = B_PER_CORE * N_ANCH * GS * GS  # 34656
# anchors in grid units (ANCHORS / stride, stride = 608 // 76 = 8)
AW = (1.25, 2.0, 4.125)
AH = (1.625, 3.75, 2.875)
LOG80 = float(np.log(np.float32(80.0)))


def _patch_tile_drain():
    """This walrus build accepts at most one sync-wait command per
    instruction; the stock TileContext tail drain carries one wait per active
    proc. Spread the waits across single-wait SP nops ahead of the drain."""
    import re
    import concourse.tile as ctile
    from concourse.vector_clock import ScopedClock, VectorClock

    if getattr(ctile.TileContext, "_drain_patched", False):
        return

    def _drain_and_barrier(self, tick_clock, wait_clock):
        gc = tick_clock.global_clock
        ticks = [int(t) for t in re.findall(r"\d+", str(gc))]
        for proc, tick in enumerate(ticks):
            if tick > 0:
                partial = VectorClock()
                partial.require_at_least(proc, tick)
                nop = self.nc.sync.nop(nofuse=True, hint="drain_wait_split")
                wait_clock.add_sem_waits(nop.ins, ScopedClock({None: partial}))
        self.nc.sync.drain()
        assert self.sems is not None
        popped = self.nc._tile_sem_poison_stack.pop()
        assert popped is self._sem_poison
        # tail barrier + sem-clear skipped: the SP wait-nops + drain already
        # guarantee completion, and the Bass preamble of every execution
        # re-clears and dma-resets the kernel sem range anyway

    ctile.TileContext._drain_and_barrier = _drain_and_barrier
    ctile.TileContext._drain_patched = True


def _install_ntff_shim():
    """Optional: lets trace=True / BASS_TRACE=1 profiling work in containers
    whose antenv package lacks axon_hooks. Harmless if unused."""
    if "antenv.axon_hooks" in sys.modules:
        return
    try:
        mod = types.ModuleType("antenv.axon_hooks")
        mod._hook = None
        mod.set_axon_ntff_profile_hook = lambda h: setattr(mod, "_hook", h)
        mod.get_axon_ntff_profile_hook = lambda: mod._hook
        sys.modules["antenv.axon_hooks"] = mod
        import antenv

        antenv.axon_hooks = mod
        from trn_agent_boot.trn_boot import _ntff_profile_via_ctypes

        mod.set_axon_ntff_profile_hook(
            _ntff_profile_via_ctypes("/opt/axon/libaxon_pjrt.so")
        )
        import concourse.bass_utils as bu

        bu.upload_artifacts = lambda tmpdir: f"local:{tmpdir}"
    except Exception:
        pass


def build_nc():
    import concourse.bass as bass
    import concourse.bacc as bacc
    import concourse.tile as tile
    from concourse import mybir

    _patch_tile_drain()

    AP = bass.AP
    f32 = mybir.dt.float32
    i32 = mybir.dt.int32
    Alu = mybir.AluOpType
    Act = mybir.ActivationFunctionType
    Ax = mybir.AxisListType
    P = G_PER_CORE  # 40 partitions of per-GT state

    nc = bacc.Bacc()
    xt_ext = nc.dram_tensor("xt", [ROWS, N_CH], f32, kind="ExternalInput")
    yt_ext = nc.dram_tensor("yt", [P, 5], f32, kind="ExternalInput")
    loss_ext = nc.dram_tensor("loss", [1, 8], f32, kind="ExternalOutput")

    with tile.TileContext(nc) as tc:
        with (
            tc.tile_pool(name="sbuf", bufs=1) as pool,
            tc.tile_pool(name="psum", bufs=1, space="PSUM") as psum,
        ):
            V = nc.vector
            # ================= one-time constants (no data deps) ==========
            iot_p = pool.tile([P, 1], i32)
            nc.gpsimd.iota(out=iot_p[:], pattern=[[0, 1]], base=0, channel_multiplier=1)
            b_i = pool.tile([P, 1], i32)  # 0 for image 0's GTs, 5776 for image 1
            V.tensor_scalar(out=b_i[:], in0=iot_p[:], scalar1=N_GT - 1,
                            scalar2=GS * GS, op0=Alu.is_gt, op1=Alu.mult)
            iota80 = pool.tile([P, N_CLS], i32)
            nc.gpsimd.iota(out=iota80[:], pattern=[[1, N_CLS]], base=0, channel_multiplier=0)
            ident = pool.tile([P, P], f32)
            nc.gpsimd.memset(ident[:], 0.0)
            nc.gpsimd.affine_select(out=ident[:], in_=ident[:], compare_op=Alu.not_equal,
                                    fill=1.0, base=0, pattern=[[-1, P]], channel_multiplier=1)
            # anchor consts, grouped layout: awh6 = (aw0,aw1,aw2, ah0,ah1,ah2)
            awh6 = pool.tile([P, 6], f32)
            rawh6 = pool.tile([P, 6], f32)  # (1/aw | 1/ah)
            # val24 groups (3 cols each): sx, sy, 5sc, tw, th | tx_t, ty_t,
            # 5*m_iou, ln(gw/aw), ln(gh/ah) | ln(sum exp), logits[cls]
            val24 = pool.tile([P, 36], f32)
            adder9 = pool.tile([P, 9], f32)  # (+1 x6 | +0.2 x3)
            V.memset(adder9[:, 0:6], 1.0)
            V.memset(adder9[:, 6:9], 0.2)
            lnp2 = pool.tile([P, 1], f32)  # ln(0.2) bias -> exp gives 0.2*e^-x
            V.memset(lnp2[:], float(np.log(np.float32(0.2))))
            for a in range(3):
                V.memset(awh6[:, a:a + 1], AW[a])
                V.memset(awh6[:, 3 + a:4 + a], AH[a])
                V.memset(rawh6[:, a:a + 1], 1.0 / AW[a])
                V.memset(rawh6[:, 3 + a:4 + a], 1.0 / AH[a])
            sg12 = pool.tile([P, 12], f32)  # (-.5 x6 | +.5 x6)
            V.memset(sg12[:, 0:6], -0.5)
            V.memset(sg12[:, 6:12], 0.5)
            gs4 = pool.tile([P, 4], f32)  # (-.5,-.5, +.5,+.5)
            V.memset(gs4[:, 0:2], -0.5)
            V.memset(gs4[:, 2:4], 0.5)
            ltab = pool.tile([P, 8], f32)
            V.memset(ltab[:, 6:8], 0.0)
            V.memset(ltab[:, 6:7], 1.0)

            # ================= load y_true shard ==========================
            yt = pool.tile([P, 5], f32)
            nc.sync.dma_start(out=yt[:], in_=yt_ext[:])

            gt4 = pool.tile([P, 4], f32)  # (gx, gy, gw, gh) in grid units
            V.tensor_scalar(out=gt4[:], in0=yt[:, 0:4], scalar1=float(GS), scalar2=None, op0=Alu.mult)
            gxy = gt4[:, 0:2]

            # floor (int cast is round-to-nearest; fix up), int-domain index math
            rne_i = pool.tile([P, 2], i32)
            V.tensor_scalar(out=rne_i[:], in0=yt[:, 0:2], scalar1=float(GS), scalar2=None, op0=Alu.mult)
            rne_f = pool.tile([P, 2], f32)
            V.tensor_copy(out=rne_f[:], in_=rne_i[:])
            fr0 = pool.tile([P, 2], f32)
            V.tensor_tensor(out=fr0[:], in0=gxy, in1=rne_f[:], op=Alu.subtract)
            neg_i = pool.tile([P, 2], i32)
            V.tensor_scalar(out=neg_i[:], in0=fr0[:], scalar1=0.0, scalar2=None, op0=Alu.is_lt)
            gij_i = pool.tile([P, 2], i32)
            V.tensor_tensor(out=gij_i[:], in0=rne_i[:], in1=neg_i[:], op=Alu.subtract)
            gij = pool.tile([P, 2], f32)
            V.tensor_copy(out=gij[:], in_=gij_i[:])
            tt = pool.tile([P, 2], f32)  # (tx_t, ty_t)
            V.tensor_tensor(out=tt[:], in0=gxy, in1=gij[:], op=Alu.subtract)
            gi = gij[:, 0:1]
            gj = gij[:, 1:2]

            cls_i = pool.tile([P, 1], i32)
            V.tensor_copy(out=cls_i[:], in_=yt[:, 4:5])

            # gather row index = b*5776 + gj*76 + gi, on device (int32)
            idx_i = pool.tile([P, 1], i32)
            V.tensor_scalar(out=idx_i[:], in0=gij_i[:, 1:2], scalar1=GS, scalar2=None, op0=Alu.mult)
            V.tensor_tensor(out=idx_i[:], in0=idx_i[:], in1=gij_i[:, 0:1], op=Alu.add)
            V.tensor_tensor(out=idx_i[:], in0=idx_i[:], in1=b_i[:], op=Alu.add)
            idx_f = pool.tile([P, 1], f32)
            V.tensor_copy(out=idx_f[:], in_=idx_i[:])
            rmix = psum.tile([P, P], f32, tag="rmix")
            nc.tensor.transpose(out=rmix[:], in_=idx_f[:, 0:1].to_broadcast([P, P]), identity=ident[:])
            mt = pool.tile([P, P], f32)  # MT[g',g] = same cell & g' later
            V.tensor_scalar(out=mt[:], in0=rmix[:], scalar1=idx_f[:, 0:1], scalar2=None, op0=Alu.is_equal)
            nc.gpsimd.affine_select(out=mt[:], in_=mt[:], compare_op=Alu.is_gt,
                                    fill=0.0, base=0, pattern=[[-1, P]], channel_multiplier=1)

            # ============ the gather: G[g, :] = xt[idx[g], :] =============
            g_t = pool.tile([P, N_CH], f32)
            nc.gpsimd.indirect_dma_start(
                out=g_t[:], out_offset=None, in_=xt_ext[:],
                in_offset=bass.IndirectOffsetOnAxis(ap=idx_i[:, 0:1], axis=0),
            )
            gv = g_t[:]

            def gview(c0, inner):  # [P, 3(anchors), inner] strided view
                base = gv[:, c0:c0 + 1]
                return AP(base.tensor, base.offset,
                          [base.ap[0], [N_ATTR, 3], [1, inner]])

            def grouped_out(dst_ap, inner):  # (a, c) -> dst col c*3+a
                return AP(dst_ap.tensor, dst_ap.offset,
                          [dst_ap.ap[0], [1, 3], [3, inner]])

            def coord_bc(ap2, ncopies):  # (v0 x n | v1 x n) coord-major bcast
                return AP(ap2.tensor, ap2.offset, [ap2.ap[0], [1, 2], [0, ncopies]])

            # ===================== activations ============================
            # ACT runs only Exp (sigmoid = 1/(1+exp(-x)) with DVE recip) and
            # one late Ln batch -> at most one visible table switch
            tmp9 = pool.tile([P, 9], f32)
            nc.scalar.activation(out=grouped_out(tmp9[:, 0:6], 2), in_=gview(0, 2), func=Act.Exp, scale=-1.0)
            nc.scalar.activation(out=tmp9[:, 6:9], in_=gview(4, 1), func=Act.Exp, scale=-1.0, bias=lnp2[:, 0:1])
            bwh6 = pool.tile([P, 6], f32)  # exp(tw|th), grouped; *anchor below
            nc.scalar.activation(out=grouped_out(bwh6[:], 2), in_=gview(2, 2), func=Act.Exp)
            V.tensor_tensor(out=tmp9[:], in0=tmp9[:], in1=adder9[:], op=Alu.add)
            V.reciprocal(out=val24[:, 0:9], in_=tmp9[:])  # sx|sy|5sc
            V.tensor_copy(out=grouped_out(val24[:, 9:15], 2), in_=gview(2, 2))  # raw tw|th
            # tx_t/ty_t as (constant-per-anchor) groups so dif is one subtract
            ttv = tt[:]
            V.tensor_copy(
                out=AP(val24[:].tensor, val24[:].offset + 15, [val24[:].ap[0], [3, 2], [1, 3]]),
                in_=AP(ttv.tensor, ttv.offset, [ttv.ap[0], [1, 2], [0, 3]]))
            q6 = pool.tile([P, 6], f32)
            V.tensor_tensor(out=q6[:], in0=coord_bc(gt4[:, 2:4], 3), in1=rawh6[:], op=Alu.mult)
            nc.scalar.activation(out=val24[:, 24:30], in_=q6[:], func=Act.Ln)

            # class-loss prep, independent of the argmax: exp/one-hot products
            # over all 3 anchors now, tiny selected sums later
            e80s = pool.tile([P, N_CLS], f32, tag="e80s")
            rs3 = pool.tile([P, 3], f32)  # sum_k exp(l[a,k]), via ACT accumulators
            for a in range(3):
                nc.scalar.activation(out=e80s[:], in_=gv[:, 5 + a * N_ATTR:85 + a * N_ATTR],
                                     func=Act.Exp, accum_out=rs3[:, a:a + 1])
            nc.scalar.activation(out=val24[:, 30:33], in_=rs3[:], func=Act.Ln)
            oh80 = pool.tile([P, N_CLS], f32)
            V.tensor_tensor(out=oh80[:], in0=iota80[:],
                            in1=cls_i[:, 0:1].to_broadcast([P, N_CLS]), op=Alu.is_equal)
            p240 = pool.tile([P, 240], f32)
            ohb = oh80[:]
            V.tensor_tensor(out=p240[:], in0=gview(5, N_CLS),
                            in1=AP(ohb.tensor, ohb.offset, [ohb.ap[0], [0, 3], [1, N_CLS]]), op=Alu.mult)
            p3v = p240[:]
            V.tensor_reduce(out=val24[:, 33:36], in_=AP(p3v.tensor, p3v.offset, [p3v.ap[0], [N_CLS, 3], [1, N_CLS]]),
                            op=Alu.add, axis=Ax.X)

            # ======================== IoU =================================
            V.tensor_tensor(out=bwh6[:], in0=bwh6[:], in1=awh6[:], op=Alu.mult)
            bxy6 = pool.tile([P, 6], f32)
            V.tensor_tensor(out=bxy6[:], in0=val24[:, 0:6], in1=coord_bc(gij[:], 3), op=Alu.add)

            def bc2(ap6, inner):  # [P, inner] -> [P, 2, inner] 0-stride bcast
                return AP(ap6.tensor, ap6.offset, [ap6.ap[0], [0, 2], [1, inner]])

            a12 = pool.tile([P, 12], f32)  # (x1,y1 | x2,y2) per anchor
            V.tensor_tensor(out=a12[:], in0=bc2(bwh6[:], 6), in1=sg12[:], op=Alu.mult)
            V.tensor_tensor(out=a12[:], in0=a12[:], in1=bc2(bxy6[:], 6), op=Alu.add)
            a1 = a12[:, 0:6]
            a2 = a12[:, 6:12]
            g12 = pool.tile([P, 4], f32)
            V.tensor_tensor(out=g12[:], in0=gs4[:], in1=AP(gt4[:].tensor, gt4[:].offset + 2, [gt4[:].ap[0], [0, 2], [1, 2]]), op=Alu.mult)
            V.tensor_tensor(out=g12[:], in0=g12[:], in1=bc2(gxy, 2), op=Alu.add)
            g1 = g12[:, 0:2]
            g2 = g12[:, 2:4]

            i1 = pool.tile([P, 6], f32)
            V.tensor_tensor(out=i1[:], in0=a1, in1=coord_bc(g1, 3), op=Alu.max)
            i2 = pool.tile([P, 6], f32)
            V.tensor_tensor(out=i2[:], in0=a2, in1=coord_bc(g2, 3), op=Alu.min)
            iwh = pool.tile([P, 6], f32)
            V.tensor_tensor(out=iwh[:], in0=i2[:], in1=i1[:], op=Alu.subtract)
            V.tensor_scalar(out=iwh[:], in0=iwh[:], scalar1=0.0, scalar2=None, op0=Alu.max)
            inter = pool.tile([P, 3], f32)
            V.tensor_tensor(out=inter[:], in0=iwh[:, 0:3], in1=iwh[:, 3:6], op=Alu.mult)

            area_a = pool.tile([P, 3], f32)
            V.tensor_tensor(out=area_a[:], in0=bwh6[:, 0:3], in1=bwh6[:, 3:6], op=Alu.mult)
            area_g = pool.tile([P, 1], f32)
            V.tensor_tensor(out=area_g[:], in0=gt4[:, 2:3], in1=gt4[:, 3:4], op=Alu.mult)
            V.tensor_scalar(out=area_g[:], in0=area_g[:], scalar1=1e-16, scalar2=None, op0=Alu.add)

            area_s = pool.tile([P, 3], f32)
            V.tensor_tensor(out=area_s[:], in0=area_a[:], in1=area_g[:, 0:1].to_broadcast([P, 3]), op=Alu.add)
            union = pool.tile([P, 3], f32)
            V.tensor_tensor(out=union[:], in0=area_s[:], in1=inter[:], op=Alu.subtract)
            runion = pool.tile([P, 3], f32)
            V.reciprocal(out=runion[:], in_=union[:])
            iou = pool.tile([P, 3], f32)
            V.tensor_tensor(out=iou[:], in0=inter[:], in1=runion[:], op=Alu.mult)

            # ============ best anchor (first-wins argmax) =================
            m_iou = pool.tile([P, 1], f32)
            V.tensor_reduce(out=m_iou[:], in_=iou[:], op=Alu.max, axis=Ax.X)
            m5 = pool.tile([P, 1], f32)
            V.tensor_scalar(out=m5[:], in0=m_iou[:], scalar1=5.0, scalar2=None, op0=Alu.mult)
            V.tensor_copy(out=val24[:, 21:24], in_=m5[:, 0:1].to_broadcast([P, 3]))
            isv = pool.tile([P, 3], f32)
            V.tensor_tensor(out=isv[:], in0=iou[:], in1=m_iou[:, 0:1].to_broadcast([P, 3]), op=Alu.is_equal)
            t01 = pool.tile([P, 1], f32)
            V.tensor_scalar(out=t01[:], in0=isv[:, 0:1], scalar1=-1.0, scalar2=1.0, op0=Alu.mult, op1=Alu.add)
            V.tensor_tensor(out=isv[:, 1:2], in0=isv[:, 1:2], in1=t01[:], op=Alu.mult)
            V.tensor_tensor(out=isv[:, 2:3], in0=t01[:], in1=isv[:, 1:2], op=Alu.subtract)

            def bc_isv(ngroups, inner):  # isv broadcast across groups/inner
                a = isv[:]
                if inner == 1:
                    return AP(a.tensor, a.offset, [a.ap[0], [0, ngroups], [1, 3]])
                return AP(a.tensor, a.offset, [a.ap[0], [1, 3], [0, inner]])

            # dedup: count later same-cell GTs with the same best anchor,
            # contracting the precomputed collision matrix against isv on PE
            psx = psum.tile([P, 3], f32, tag="psx")
            nc.tensor.matmul(out=psx[:], lhsT=mt[:], rhs=isv[:], start=True, stop=True)
            k3 = pool.tile([P, 3], f32, tag="k3")
            V.tensor_tensor(out=k3[:], in0=psx[:], in1=isv[:], op=Alu.mult)
            kil = pool.tile([P, 1], f32)
            V.tensor_reduce(out=kil[:], in_=k3[:], op=Alu.add, axis=Ax.X)
            keep = pool.tile([P, 1], f32)
            V.tensor_scalar(out=keep[:], in0=kil[:], scalar1=0.0, scalar2=None, op0=Alu.is_equal)

            # ======= select best-anchor values: 11 groups at once =========
            # (targets / lse / pick were precomputed per anchor, so no ACT
            # work remains after the argmax)
            selp = pool.tile([P, 36], f32)
            V.tensor_tensor(out=selp[:], in0=val24[:], in1=bc_isv(12, 1), op=Alu.mult)
            selr = pool.tile([P, 12], f32)
            sp = selp[:]
            V.tensor_reduce(out=selr[:], in_=AP(sp.tensor, sp.offset, [sp.ap[0], [3, 12], [1, 3]]),
                            op=Alu.add, axis=Ax.X)
            # cols: 0 sx, 1 sy, 2 5sc, 3 tw, 4 th | 5 tx_t, 6 ty_t, 7 5miou,
            #       8 tw_t, 9 th_t | 10 lse, 11 pick

            # ====== per-GT loss columns (x,y,conf,w,h | cls | count) ======
            dif5 = pool.tile([P, 5], f32)
            V.tensor_tensor(out=dif5[:], in0=selr[:, 0:5], in1=selr[:, 5:10], op=Alu.subtract)
            V.tensor_tensor(out=ltab[:, 0:5], in0=dif5[:], in1=dif5[:], op=Alu.mult)
            V.tensor_tensor(out=ltab[:, 5:6], in0=selr[:, 10:11], in1=selr[:, 11:12], op=Alu.subtract)

            # ====== reduce over GTs via PE (keep as lhsT applies the
            # duplicate mask during the contraction), finalize cls term =====
            ps = psum.tile([1, 8], f32)
            nc.tensor.matmul(out=ps[:], lhsT=keep[:], rhs=ltab[:], start=True, stop=True)
            o8 = pool.tile([1, 8], f32)
            V.tensor_copy(out=o8[:], in_=ps[:])
            nc.sync.dma_start(out=loss_ext[:], in_=o8[:])

    nc.finalize()
    return nc


_NC_CACHE = None
LAST_RESULTS = None


def _get_nc():
    global _NC_CACHE
    if _NC_CACHE is None:
        _NC_CACHE = build_nc()
    return _NC_CACHE


def make_in_maps(x, y_true):
    x = np.asarray(x, dtype=np.float32)
    y = np.asarray(y_true, dtype=np.float32)
    in_maps = []
    for c in range(N_CORES):
        xs = np.ascontiguousarray(
            x[c * B_PER_CORE:(c + 1) * B_PER_CORE].transpose(0, 2, 3, 1)
        ).reshape(ROWS, N_CH)
        ys = np.ascontiguousarray(
            y[c * B_PER_CORE:(c + 1) * B_PER_CORE].reshape(G_PER_CORE, 5)
        )
        in_maps.append({"xt": xs, "yt": ys})
    return in_maps


def kernel(x, y_true):
    global LAST_RESULTS
    _install_ntff_shim()
    from concourse.bass_utils import run_bass_kernel_spmd

    nc = _get_nc()
    br = run_bass_kernel_spmd(
        nc, make_in_maps(x, y_true), list(range(N_CORES))
    )
    LAST_RESULTS = br
    return finalize_partials([r["loss"][0] for r in br.results])


def finalize_partials(parts):
    """parts: per-core [8] = (lx, ly, lw, lh, cls_obj, lconf, n_obj, 0)."""
    acc = np.zeros(6, np.float32)
    l80 = np.float32(LOG80)
    for p in parts:
        p = np.asarray(p, np.float32)
        tcl = np.float32(p[6] * -l80 + np.float32(CELLS_PER_CORE * LOG80))
        acc[0] += p[0]
        acc[1] += p[1]
        acc[5] += p[2]
        acc[2] += p[3]
        acc[3] += p[4]
        acc[4] += np.float32(p[5] + tcl)
    return acc

